# revision 20
# baseline (speedup 1.0000x reference)
"""AffinityLoss forward on 8 Trainium2 NeuronCores (Bass/Tile).

Math (per batch sample b):
    s      = sum_n Y[n]                        (D2,)
    YY1[n] = Y[n] . s                          (row sums of Y Y^T)
    d      = 1/sqrt(YY1 + eps)
    loss_b = ||V^T D V||_F^2 + ||Y^T D Y||_F^2 - 2 ||V^T D Y||_F^2
    out    = mean_b loss_b

Sharding: data-parallel over batch -- core b computes loss_b for its own
(V[b], Y[b]); the host means the 8 scalars.

Per-core plan:
  - Y (16384x64) is DMA-cast fp32->bf16 and kept resident in SBUF as
    [128 partitions, 128 chunks, 64].
  - s: DVE reduce over chunks, then a ones-matmul does the partition sum
    AND broadcasts s to all 128 partitions in one PE op.
  - YY1: DVE multiply (broadcast s along chunks) + free-dim reduce;
    d = reciprocal(ACT sqrt(YY1 + eps)).
  - V streams in 2MB slabs (DMA-cast bf16). Per 128-row chunk: scale the
    V/Y rows by d (lhsT side only), then 5 PE matmuls accumulate the Gram
    blocks into 5 PSUM banks (fp32) across all 128 chunks:
        pA = (dV0)^T V   [128,256]   pB = (dV0)^T Y [128,64]
        pC = (dV1)^T V1  [128,128]   pD = (dV1)^T Y [128,64]
        pE = (dY)^T  Y   [64,64]
    (V0 = V cols 0:128, V1 = cols 128:256; the VDV block (1,0) is skipped
    by symmetry and pA's (0,1) block counted twice.)
  - Epilogue: ACT Square with accum_out per block, weighted combine on
    DVE, ones-matmul partition reduction -> scalar -> HBM.
"""

import numpy as np

B, N, D1, D2 = 8, 16384, 256, 64
P = 128
EPS = 1e-12

_NC_CACHE = {}


def _hoist_extra_waits(nc):
    """Split multi-wait instructions: keep one sem wait on the instruction,
    hoist the rest onto standalone EventSemaphore ops just before it.

    The walrus build in this container rejects >1 sync wait per lowered
    instruction ("Too many sync wait commands"), while Tile freely attaches
    several. Standalone waits on the same engine are semantically identical
    (engine streams are serial and Tile sems are monotone)."""
    import concourse.mybir as mybir

    n_hoisted = 0
    for fn in nc.m.functions:
        for bb in fn.blocks:
            out = []
            changed = False
            for inst in bb.instructions:
                si = inst.sync_info
                waits = list(si.on_wait) if si is not None and si.on_wait else []
                if len(waits) > 1:
                    for k, w in enumerate(waits[:-1]):
                        ev = mybir.InstEventSemaphore(
                            name=f"{inst.name}-xw{k}", ins=[], outs=[])
                        ev.engine = inst.engine
                        ev.sync_info = mybir.SyncInfo(on_wait=[w], on_update=[])
                        out.append(ev)
                        n_hoisted += 1
                    inst.sync_info = mybir.SyncInfo(
                        on_wait=[waits[-1]], on_update=list(si.on_update))
                    changed = True
                out.append(inst)
            if changed:
                bb.instructions = out
    return n_hoisted


def _build(n=N, mm="bf16", hoist=True, loop=1, dma="swdge"):
    """Build the per-core Bass program. mm in {"bf16", "f32r", "f32"}.

    hoist=True runs _hoist_extra_waits (needed for the walrus HW compile,
    but CoreSim's race detector can't consume the hoisted instructions).
    loop>1 wraps the whole body in a hardware For_i that repeats the full
    computation -- used only for wall-clock timing amplification.

    dma="swdge": inputs are cast fp32->bf16 in-flight (gpsimd DMA) and the
    lhsT side is scaled by the full d. dma="hwdge": inputs stay fp32 in
    SBUF (sync DMA), both matmul operands are one q-scaled bf16 tile with
    q = d^(1/2); this variant contains no SWDGE ops, which For_i timing
    loops require (the SWDGE loop-reset ISA op is unsupported here)."""
    import concourse.bass as bass
    import concourse.mybir as mybir
    import concourse.tile as tile
    from concourse.tile_rust import add_dep_helper

    f32 = mybir.dt.float32
    bf16 = mybir.dt.bfloat16
    dt_data = f32 if mm in ("f32", "f32r") else bf16
    AF = mybir.ActivationFunctionType
    ALU = mybir.AluOpType
    AX = mybir.AxisListType

    nchunks = n // P
    slab = min(16, nchunks)          # chunks per V slab (16 -> 2MB fp32 reads)
    nslabs = nchunks // slab
    grp = min(32, nchunks)           # chunks per YY1 group
    ngrps = nchunks // grp

    nc = bass.Bass()
    v = nc.dram_tensor("input", [n, D1], f32, kind="ExternalInput")
    y = nc.dram_tensor("target", [n, D2], f32, kind="ExternalInput")
    loss_out = nc.dram_tensor("loss", [1, 1], f32, kind="ExternalOutput")

    y_re = y[:, :].rearrange("(c p) d -> p c d", p=P)
    v_re = v[:, :].rearrange("(s c p) i -> s p c i", c=slab, p=P)

    import contextlib

    with tile.TileContext(nc) as tc:
        with (
            tc.tile_pool(name="singles", bufs=1) as singles,
            tc.tile_pool(name="vslabs", bufs=3) as vslabs,
            tc.tile_pool(name="wts", bufs=6) as wts,
            tc.tile_pool(name="scr", bufs=2) as scr,
            tc.tile_pool(name="psum", bufs=1, space="PSUM") as psum,
            tc.For_i(0, loop, 1) if loop > 1 else contextlib.nullcontext(),
        ):
            ones = singles.tile([P, P], f32)
            nc.vector.memset(ones[:], 1.0)
            epsc = singles.tile([P, 1], f32)
            nc.vector.memset(epsc[:], EPS)

            hw = dma == "hwdge"
            dmae = nc.sync if hw else nc.gpsimd
            dt_raw = f32 if hw else dt_data

            # ---- Y resident load (swdge: cast bf16 in-flight), in halves ----
            y_sb = singles.tile([P, nchunks, D2], dt_raw)
            h = nchunks // 2
            ydma0 = dmae.dma_start(y_sb[:, 0:h], y_re[:, 0:h])
            ydma1 = dmae.dma_start(y_sb[:, h:nchunks], y_re[:, h:nchunks])
            # chain the halves so "after ydma1" implies all of Y has landed
            add_dep_helper(ydma1.ins, ydma0.ins, sync=True, reason="Y chain")

            # ---- s = colsum(Y): reduce chunks on DVE, partitions on PE ----
            cs = singles.tile([P, 2, D2], f32)
            nc.vector.tensor_reduce(
                out=cs[:, 0], in_=y_sb[:, 0:h].rearrange("p c d -> p d c"),
                axis=AX.X, op=ALU.add)
            nc.vector.tensor_reduce(
                out=cs[:, 1], in_=y_sb[:, h:nchunks].rearrange("p c d -> p d c"),
                axis=AX.X, op=ALU.add)
            cst = singles.tile([P, D2], f32)
            nc.vector.tensor_tensor(cst[:], cs[:, 0], cs[:, 1], ALU.add)
            # ones^T @ cst sums partitions AND broadcasts s to all partitions
            psum_s = psum.tile([P, D2], f32)
            nc.tensor.matmul(psum_s[:], ones[:], cst[:], start=True, stop=True)
            s_b = singles.tile([P, D2], dt_raw)
            nc.vector.tensor_copy(s_b[:], psum_s[:])

            # ---- YY1; scale factor d (swdge, one side) or q=d^(1/2) (hwdge,
            # both sides), in groups of `grp` chunks ----
            yy1 = singles.tile([P, nchunks], f32)
            d_all = singles.tile([P, nchunks], f32)
            for g in range(ngrps):
                sl = slice(g * grp, (g + 1) * grp)
                prod = scr.tile([P, grp, D2], dt_data, tag="prod")
                nc.vector.tensor_tensor(
                    prod[:], y_sb[:, sl],
                    s_b[:, None, :].to_broadcast((P, grp, D2)), ALU.mult)
                nc.vector.tensor_reduce(
                    out=yy1[:, sl], in_=prod[:], axis=AX.X, op=ALU.add)
                sq = scr.tile([P, grp], f32, tag="sq")
                nc.scalar.activation(sq[:], yy1[:, sl], AF.Sqrt, bias=epsc[:])
                if hw:
                    sq2 = scr.tile([P, grp], f32, tag="sq2")
                    nc.scalar.activation(sq2[:], sq[:], AF.Sqrt)
                    nc.vector.reciprocal(d_all[:, sl], sq2[:])
                else:
                    nc.vector.reciprocal(d_all[:, sl], sq[:])

            # ---- Gram accumulators ----
            pA = psum.tile([P, D1], f32)       # VDV rows 0:128, all cols
            pB = psum.tile([P, D2], f32)       # VDY rows 0:128
            pC = psum.tile([P, D1 - P], f32)   # VDV rows 128:256, cols 128:256
            pD = psum.tile([P, D2], f32)       # VDY rows 128:256
            pE = psum.tile([D2, D2], f32)      # YDY

            # ---- V stream + matmuls ----
            mm_cast = (lambda ap: ap.bitcast(mybir.dt.float32r)) if mm == "f32r" \
                else (lambda ap: ap)
            for si in range(nslabs):
                vtile = vslabs.tile([P, slab, D1], dt_raw, tag="vslab")
                vdma = dmae.dma_start(vtile[:], v_re[si])
                if si == 0:
                    # keep V off the HBM port until Y has fully landed
                    add_dep_helper(vdma.ins, ydma1.ins, sync=True,
                                   reason="Y before V")
                for cc in range(slab):
                    c = si * slab + cc
                    st, sp = (c == 0), (c == nchunks - 1)
                    dcol = d_all[:, c:c + 1]
                    vdt = wts.tile([P, D1], dt_data, tag="vd")
                    ydt = wts.tile([P, D2], dt_data, tag="yd")
                    nc.vector.tensor_scalar_mul(vdt[:], vtile[:, cc], dcol)
                    if hw:
                        # ACT does the Y-row scale+cast; DVE is the busier engine
                        nc.scalar.mul(ydt[:], y_sb[:, c], dcol)
                        rv, ry = vdt[:], ydt[:]
                    else:
                        nc.vector.tensor_scalar_mul(ydt[:], y_sb[:, c], dcol)
                        rv, ry = vtile[:, cc], y_sb[:, c]
                    mc = mm_cast
                    nc.tensor.matmul(pA[:], mc(vdt[:, 0:P]), mc(rv),
                                     start=st, stop=sp)
                    nc.tensor.matmul(pB[:], mc(vdt[:, 0:P]), mc(ry),
                                     start=st, stop=sp)
                    nc.tensor.matmul(pC[:], mc(vdt[:, P:D1]),
                                     mc(rv[:, P:D1] if hw else vtile[:, cc, P:D1]),
                                     start=st, stop=sp)
                    nc.tensor.matmul(pD[:], mc(vdt[:, P:D1]), mc(ry),
                                     start=st, stop=sp)
                    nc.tensor.matmul(pE[:], mc(ydt[:]), mc(ry),
                                     start=st, stop=sp)

            # ---- loss = 1*A^2 + 2*B^2 + 1*C^2 + 1*E^2 - 2*(VDY)^2 ----
            acc = singles.tile([P, 6], f32)
            nc.vector.memset(acc[:], 0.0)
            regions = [
                (pA[:, 0:P], 0),      # A  : VDV[0:128, 0:128]      w=+1
                (pA[:, P:D1], 1),     # B  : VDV[0:128, 128:256]    w=+2
                (pC[:], 2),           # C  : VDV[128:256, 128:256]  w=+1
                (pB[:], 3),           # VDY upper                   w=-2
                (pD[:], 4),           # VDY lower                   w=-2
                (pE[:], 5),           # YDY                         w=+1
            ]
            for ap, col in regions:
                pr = ap.shape[0]
                fr = int(np.prod(ap.shape[1:]))
                sqo = scr.tile([P, D1], f32, tag="sqo")
                nc.scalar.activation(sqo[:pr, :fr], ap, AF.Square,
                                     accum_out=acc[:pr, col:col + 1])

            t0 = singles.tile([P, 1], f32)
            t1 = singles.tile([P, 1], f32)
            wv = singles.tile([P, 1], f32)
            # t0 = A + C + E_y + 2*B
            nc.vector.tensor_tensor(t0[:], acc[:, 0:1], acc[:, 2:3], ALU.add)
            nc.vector.tensor_tensor(t0[:], t0[:], acc[:, 5:6], ALU.add)
            nc.vector.tensor_scalar_mul(t1[:], acc[:, 1:2], 2.0)
            nc.vector.tensor_tensor(t0[:], t0[:], t1[:], ALU.add)
            # t1 = -2 * (VDY_u + VDY_l)
            nc.vector.tensor_tensor(t1[:], acc[:, 3:4], acc[:, 4:5], ALU.add)
            nc.vector.tensor_scalar_mul(t1[:], t1[:], -2.0)
            nc.vector.tensor_tensor(wv[:], t0[:], t1[:], ALU.add)

            pl = psum.tile([1, 1], f32)
            nc.tensor.matmul(pl[:], wv[:], ones[:, 0:1], start=True, stop=True)
            lres = singles.tile([1, 1], f32)
            nc.vector.tensor_copy(lres[:], pl[:])
            nc.sync.dma_start(loss_out[:, :], lres[:])

    if hoist:
        _hoist_extra_waits(nc)
    return nc


def _get_nc(n=N, mm="bf16", dma="swdge"):
    key = (n, mm, dma)
    if key not in _NC_CACHE:
        _NC_CACHE[key] = _build(n=n, mm=mm, dma=dma)
    return _NC_CACHE[key]


def run_spmd(input, target, mm="bf16", dma="swdge", **run_kwargs):
    """Run on 8 cores (one batch sample each); returns (loss, results)."""
    from concourse.bass_utils import run_bass_kernel_spmd

    nb = input.shape[0]
    nc = _get_nc(n=input.shape[1], mm=mm, dma=dma)
    in_maps = [
        {
            "input": np.ascontiguousarray(input[b], dtype=np.float32),
            "target": np.ascontiguousarray(target[b], dtype=np.float32),
        }
        for b in range(nb)
    ]
    res = run_bass_kernel_spmd(nc, in_maps, core_ids=list(range(nb)), **run_kwargs)
    losses = np.array([r["loss"][0, 0] for r in res.results], dtype=np.float32)
    return np.asarray(losses.mean(), dtype=np.float32), res


def kernel(input, target):
    loss, _ = run_spmd(np.asarray(input), np.asarray(target))
    return loss


# revision 39
# speedup vs baseline: 1.5561x; 1.5561x over previous
"""AffinityLoss forward on 8 Trainium2 NeuronCores (Bass/Tile).

Math (per batch sample b):
    s      = sum_n Y[n]                        (D2,)
    YY1[n] = Y[n] . s                          (row sums of Y Y^T)
    d      = 1/sqrt(YY1 + eps)
    loss_b = ||V^T D V||_F^2 + ||Y^T D Y||_F^2 - 2 ||V^T D Y||_F^2
    out    = mean_b loss_b

Sharding: data-parallel over batch -- core b computes loss_b for its own
(V[b], Y[b]); the host means the 8 scalars.

Per-core plan (default dma="hwdge" path):
  - Rows are mapped partition-major (partition p holds 128 consecutive
    HBM rows) so every DMA moves long contiguous runs per partition;
    any 128-row grouping is valid for the N-contraction.
  - Y (16384x64) loads fp32 in 4 quarters; a per-quarter DVE reduce
    pipelines the column-sum s behind the DMA; a ones-matmul then does
    the partition sum AND broadcasts s to all partitions in one PE op.
  - Per 32-chunk group (fused into the V slab loop): YY1 = DVE multiply
    (broadcast s along chunks) + free-dim reduce; scale factor
    q = (YY1+eps)^(-1/4) via two ACT sqrts + DVE reciprocal.
  - V streams in 4MB fp32 slabs. Per 128-row chunk: DVE scales+casts V
    rows by q into bf16 (ACT does the Y rows), then 5 PE matmuls
    accumulate the Gram blocks into 5 PSUM banks (fp32) over all 128
    chunks:
        pA = Vq0^T Vq   [128,256]   pB = Vq0^T Yq [128,64]
        pC = Vq1^T Vq1  [128,128]   pD = Vq1^T Yq [128,64]
        pE = Yq^T  Yq   [64,64]
    (Vq0 = scaled V cols 0:128, Vq1 = cols 128:256; the VDV block (1,0)
    is skipped by symmetry and pA's (0,1) block counted twice.)
  - Epilogue: ACT Square with accum_out per block, weighted combine on
    DVE, ones-matmul partition reduction -> scalar -> HBM.
"""

import numpy as np

B, N, D1, D2 = 8, 16384, 256, 64
P = 128
EPS = 1e-12

_NC_CACHE = {}


def _hoist_extra_waits(nc):
    """Split multi-wait instructions: keep one sem wait on the instruction,
    hoist the rest onto standalone EventSemaphore ops just before it.

    The walrus build in this container rejects >1 sync wait per lowered
    instruction ("Too many sync wait commands"), while Tile freely attaches
    several. Standalone waits on the same engine are semantically identical
    (engine streams are serial and Tile sems are monotone)."""
    import concourse.mybir as mybir

    n_hoisted = 0
    for fn in nc.m.functions:
        for bb in fn.blocks:
            out = []
            changed = False
            for inst in bb.instructions:
                si = inst.sync_info
                waits = list(si.on_wait) if si is not None and si.on_wait else []
                if len(waits) > 1:
                    for k, w in enumerate(waits[:-1]):
                        ev = mybir.InstEventSemaphore(
                            name=f"{inst.name}-xw{k}", ins=[], outs=[])
                        ev.engine = inst.engine
                        ev.sync_info = mybir.SyncInfo(on_wait=[w], on_update=[])
                        out.append(ev)
                        n_hoisted += 1
                    inst.sync_info = mybir.SyncInfo(
                        on_wait=[waits[-1]], on_update=list(si.on_update))
                    changed = True
                out.append(inst)
            if changed:
                bb.instructions = out
    return n_hoisted


def _build(n=N, mm="bf16", hoist=True, loop=1, dma="swdge", variant="full"):
    """Build the per-core Bass program. mm in {"bf16", "f32r", "f32"}.

    hoist=True runs _hoist_extra_waits (needed for the walrus HW compile,
    but CoreSim's race detector can't consume the hoisted instructions).
    loop>1 wraps the whole body in a hardware For_i that repeats the full
    computation -- used only for wall-clock timing amplification.

    dma="swdge": inputs are cast fp32->bf16 in-flight (gpsimd DMA) and the
    lhsT side is scaled by the full d. dma="hwdge": inputs stay fp32 in
    SBUF (sync DMA), both matmul operands are one q-scaled bf16 tile with
    q = d^(1/2); this variant contains no SWDGE ops, which For_i timing
    loops require (the SWDGE loop-reset ISA op is unsupported here).

    variant: "full" | "dma_only" (just the input streaming, for isolating
    HBM bandwidth) | "no_yy1" (constant d, no stats prologue)."""
    import concourse.bass as bass
    import concourse.mybir as mybir
    import concourse.tile as tile
    from concourse.tile_rust import add_dep_helper

    f32 = mybir.dt.float32
    bf16 = mybir.dt.bfloat16
    dt_data = f32 if mm in ("f32", "f32r") else bf16
    AF = mybir.ActivationFunctionType
    ALU = mybir.AluOpType
    AX = mybir.AxisListType

    nchunks = n // P
    slab = min(32, nchunks)          # chunks per V slab (32 -> 4MB fp32 reads)
    nslabs = nchunks // slab
    nq = min(4, nchunks)             # Y load quarters (colsum pipelines behind)
    qsz = nchunks // nq

    nc = bass.Bass()
    v = nc.dram_tensor("input", [n, D1], f32, kind="ExternalInput")
    y = nc.dram_tensor("target", [n, D2], f32, kind="ExternalInput")
    loss_out = nc.dram_tensor("loss", [1, 1], f32, kind="ExternalOutput")

    # Partition-major row mapping: partition p holds rows [p*nchunks,
    # (p+1)*nchunks); "chunk" c = {row p*nchunks+c for all p}. Any 128-row
    # grouping is valid for the contraction (and d is computed in the same
    # (p, c) layout), and this one gives each partition long CONTIGUOUS HBM
    # runs (32KB per V slab) instead of one-row (1KB / 256B) descriptors.
    y_re = y[:, :].rearrange("(p c) d -> p c d", c=nchunks)
    v_re = v[:, :].rearrange("(p c) i -> p c i", c=nchunks)

    import contextlib

    with tile.TileContext(nc) as tc:
        with (
            tc.tile_pool(name="singles", bufs=1) as singles,
            tc.tile_pool(name="vslabs", bufs=3) as vslabs,
            tc.tile_pool(name="wts", bufs=6) as wts,
            tc.tile_pool(name="scr", bufs=2) as scr,
            tc.tile_pool(name="psum", bufs=1, space="PSUM") as psum,
            tc.For_i(0, loop, 1) if loop > 1 else contextlib.nullcontext(),
        ):
            ones = singles.tile([P, P], f32)
            nc.vector.memset(ones[:], 1.0)
            epsc = singles.tile([P, 1], f32)
            nc.vector.memset(epsc[:], EPS)

            hw = dma == "hwdge"
            dmae = nc.sync if hw else nc.gpsimd
            dt_raw = f32 if hw else dt_data

            # ---- Y resident load (swdge: cast bf16 in-flight), quarters ----
            y_sb = singles.tile([P, nchunks, D2], dt_raw)
            ydmas = []
            for qi in range(nq):
                qs = slice(qi * qsz, (qi + 1) * qsz)
                ydmas.append(dmae.dma_start(y_sb[:, qs], y_re[:, qs]))

            do_stats = variant == "full"
            do_mm = variant != "dma_only"

            if variant == "no_yy1":
                d_all = singles.tile([P, nchunks], f32)
                nc.vector.memset(d_all[:], 0.002)

            # ---- s = colsum(Y): per-quarter reduces (pipelined behind the
            # Y DMA quarters), then the partition sum + broadcast on PE ----
            s_done = None
            if do_stats:
                cs = singles.tile([P, nq, D2], f32)
                for qi in range(nq):
                    qs = slice(qi * qsz, (qi + 1) * qsz)
                    nc.vector.tensor_reduce(
                        out=cs[:, qi],
                        in_=y_sb[:, qs].rearrange("p c d -> p d c"),
                        axis=AX.X, op=ALU.add)
                cst = singles.tile([P, D2], f32)
                nc.vector.tensor_tensor(cst[:], cs[:, 0], cs[:, 1], ALU.add)
                for qi in range(2, nq):
                    nc.vector.tensor_tensor(cst[:], cst[:], cs[:, qi], ALU.add)
                # ones^T @ cst sums partitions AND broadcasts s to all of them
                psum_s = psum.tile([P, D2], f32)
                nc.tensor.matmul(psum_s[:], ones[:], cst[:], start=True,
                                 stop=True)
                s_b = singles.tile([P, D2], dt_raw)
                s_done = nc.vector.tensor_copy(s_b[:], psum_s[:])

                yy1 = singles.tile([P, nchunks], f32)
                d_all = singles.tile([P, nchunks], f32)

            def emit_yy1_group(sl):
                """YY1 + scale factor for chunk range sl: d (swdge, one side)
                or q=d^(1/2) (hwdge, both sides)."""
                gw = sl.stop - sl.start
                prod = scr.tile([P, gw, D2], dt_data, tag="prod")
                nc.vector.tensor_tensor(
                    prod[:], y_sb[:, sl],
                    s_b[:, None, :].to_broadcast((P, gw, D2)), ALU.mult)
                nc.vector.tensor_reduce(
                    out=yy1[:, sl], in_=prod[:], axis=AX.X, op=ALU.add)
                sq = scr.tile([P, gw], f32, tag="sq")
                nc.scalar.activation(sq[:], yy1[:, sl], AF.Sqrt, bias=epsc[:])
                if hw:
                    sq2 = scr.tile([P, gw], f32, tag="sq2")
                    nc.scalar.activation(sq2[:], sq[:], AF.Sqrt)
                    nc.vector.reciprocal(d_all[:, sl], sq2[:])
                else:
                    nc.vector.reciprocal(d_all[:, sl], sq[:])

            # ---- Gram accumulators ----
            if do_mm:
                pA = psum.tile([P, D1], f32)      # VDV rows 0:128, all cols
                pB = psum.tile([P, D2], f32)      # VDY rows 0:128
                pC = psum.tile([P, D1 - P], f32)  # VDV rows 128:, cols 128:
                pD = psum.tile([P, D2], f32)      # VDY rows 128:256
                pE = psum.tile([D2, D2], f32)     # YDY

            # ---- V stream + matmuls ----
            mm_cast = (lambda ap: ap.bitcast(mybir.dt.float32r)) if mm == "f32r" \
                else (lambda ap: ap)
            for si in range(nslabs):
                vtile = vslabs.tile([P, slab, D1], dt_raw, tag="vslab")
                vdma = dmae.dma_start(vtile[:],
                                      v_re[:, si * slab:(si + 1) * slab])
                if si == 0:
                    # keep V mostly off the HBM port until Y has landed (the
                    # quarters run concurrently and finish together)
                    add_dep_helper(vdma.ins, ydmas[-1].ins, sync=True,
                                   reason="Y before V")
                if do_stats:
                    emit_yy1_group(slice(si * slab, (si + 1) * slab))
                for cc in range(slab):
                    if not do_mm:
                        break
                    c = si * slab + cc
                    st, sp = (c == 0), (c == nchunks - 1)
                    dcol = d_all[:, c:c + 1]
                    vdt = wts.tile([P, D1], dt_data, tag="vd")
                    ydt = wts.tile([P, D2], dt_data, tag="yd")
                    nc.vector.tensor_scalar_mul(vdt[:], vtile[:, cc], dcol)
                    if hw:
                        # ACT does the Y-row scale+cast; DVE is the busier engine
                        nc.scalar.mul(ydt[:], y_sb[:, c], dcol)
                        rv, ry = vdt[:], ydt[:]
                    else:
                        nc.vector.tensor_scalar_mul(ydt[:], y_sb[:, c], dcol)
                        rv, ry = vtile[:, cc], y_sb[:, c]
                    mc = mm_cast
                    nc.tensor.matmul(pA[:], mc(vdt[:, 0:P]), mc(rv),
                                     start=st, stop=sp)
                    nc.tensor.matmul(pB[:], mc(vdt[:, 0:P]), mc(ry),
                                     start=st, stop=sp)
                    nc.tensor.matmul(pC[:], mc(vdt[:, P:D1]),
                                     mc(rv[:, P:D1] if hw else vtile[:, cc, P:D1]),
                                     start=st, stop=sp)
                    nc.tensor.matmul(pD[:], mc(vdt[:, P:D1]), mc(ry),
                                     start=st, stop=sp)
                    nc.tensor.matmul(pE[:], mc(ydt[:]), mc(ry),
                                     start=st, stop=sp)

            if not do_mm:
                # dma_only: consume a token from each stream so nothing is
                # dead, then write it out
                lres = singles.tile([1, 1], f32)
                tok = singles.tile([1, 2], f32)
                nc.vector.tensor_copy(tok[:, 0:1], vtile[0:1, 0, 0:1])
                nc.vector.tensor_copy(tok[:, 1:2], y_sb[0:1, 0, 0:1])
                nc.vector.tensor_tensor(lres[:], tok[:, 0:1], tok[:, 1:2],
                                        ALU.add)
                nc.sync.dma_start(loss_out[:, :], lres[:])

            # ---- loss = 1*A^2 + 2*B^2 + 1*C^2 + 1*E^2 - 2*(VDY)^2 ----
            if not do_mm:
                regions = []
            else:
                acc = singles.tile([P, 6], f32)
                nc.vector.memset(acc[:], 0.0)
                regions = [
                    (pA[:, 0:P], 0),  # A  : VDV[0:128, 0:128]      w=+1
                    (pA[:, P:D1], 1),  # B : VDV[0:128, 128:256]    w=+2
                    (pC[:], 2),       # C  : VDV[128:256, 128:256]  w=+1
                    (pB[:], 3),       # VDY upper                   w=-2
                    (pD[:], 4),       # VDY lower                   w=-2
                    (pE[:], 5),       # YDY                         w=+1
                ]
            for ap, col in regions:
                pr = ap.shape[0]
                fr = int(np.prod(ap.shape[1:]))
                sqo = scr.tile([P, D1], f32, tag="sqo")
                nc.scalar.activation(sqo[:pr, :fr], ap, AF.Square,
                                     accum_out=acc[:pr, col:col + 1])

            if do_mm:
                t0 = singles.tile([P, 1], f32)
                t1 = singles.tile([P, 1], f32)
                wv = singles.tile([P, 1], f32)
                # t0 = A + C + E_y + 2*B
                nc.vector.tensor_tensor(t0[:], acc[:, 0:1], acc[:, 2:3],
                                        ALU.add)
                nc.vector.tensor_tensor(t0[:], t0[:], acc[:, 5:6], ALU.add)
                nc.vector.tensor_scalar_mul(t1[:], acc[:, 1:2], 2.0)
                nc.vector.tensor_tensor(t0[:], t0[:], t1[:], ALU.add)
                # t1 = -2 * (VDY_u + VDY_l)
                nc.vector.tensor_tensor(t1[:], acc[:, 3:4], acc[:, 4:5],
                                        ALU.add)
                nc.vector.tensor_scalar_mul(t1[:], t1[:], -2.0)
                nc.vector.tensor_tensor(wv[:], t0[:], t1[:], ALU.add)

                pl = psum.tile([1, 1], f32)
                nc.tensor.matmul(pl[:], wv[:], ones[:, 0:1], start=True,
                                 stop=True)
                lres = singles.tile([1, 1], f32)
                nc.vector.tensor_copy(lres[:], pl[:])
                nc.sync.dma_start(loss_out[:, :], lres[:])

    if hoist:
        _hoist_extra_waits(nc)
    return nc


def _get_nc(n=N, mm="bf16", dma="hwdge"):
    key = (n, mm, dma)
    if key not in _NC_CACHE:
        _NC_CACHE[key] = _build(n=n, mm=mm, dma=dma)
    return _NC_CACHE[key]


def run_spmd(input, target, mm="bf16", dma="hwdge", **run_kwargs):
    """Run on 8 cores (one batch sample each); returns (loss, results)."""
    from concourse.bass_utils import run_bass_kernel_spmd

    nb = input.shape[0]
    nc = _get_nc(n=input.shape[1], mm=mm, dma=dma)
    in_maps = [
        {
            "input": np.ascontiguousarray(input[b], dtype=np.float32),
            "target": np.ascontiguousarray(target[b], dtype=np.float32),
        }
        for b in range(nb)
    ]
    res = run_bass_kernel_spmd(nc, in_maps, core_ids=list(range(nb)), **run_kwargs)
    losses = np.array([r["loss"][0, 0] for r in res.results], dtype=np.float32)
    return np.asarray(losses.mean(), dtype=np.float32), res


def kernel(input, target):
    loss, _ = run_spmd(np.asarray(input), np.asarray(target))
    return loss
